# revision 11
# baseline (speedup 1.0000x reference)
"""Circulant 1x1 conv (nn_Circulant1x1Conv) as a Trainium2 Bass kernel.

Math: the reference does, per spatial position r (N = batch*h*w rows):
    y[r, s*C + n] = irfft(rfft(x[r, :]) * cf[s])[n]  (circular convolution)
which is exactly a matmul  Y(N, 2048) = X(N, 512) @ W(512, 2048)  with
    W[k, s*C + n] = c_s[(n - k) mod C],   c_s = irfft(cf[s], n=C).

Crucially the native memory layouts are already transposed the right way:
  x[b] viewed as (C=512, h*w=1024) is X^T for that batch, and the output
  (nstack*C=2048, h*w) per batch is Y^T. So per batch:
      Out_b (2048, hw) = W^T @ X_b  ==  matmul(out, lhsT=W, rhs=X_b)
  on the tensor engine with zero data transposes anywhere.

Sharding: data-parallel over batch, 4 batches per core x 8 cores. Each core
computes a (2048, 4096) = (512, 2048)^T @ (512, 4096) matmul.

Precision knob DT_KIND:
  - "f32r": fp32 data, PE in fp32r (replicated/TF32-like) mode: 1 cycle/row
            at free-dim >= 256 per the cost model -> bf16-speed w/ fp32 inputs.
  - "bf16": inputs cast to bf16 on host; ~5e-3 rel error.
  - "f32":  exact fp32 matmul, 4 cycles/row (slow; debugging fallback).
"""

import numpy as np

SIZE = 512          # channels C (circulant size)
NSTACK = 4
BATCH = 32
HW = 32 * 32
N_CORES = 8
BPC = BATCH // N_CORES          # batches per core = 4
COLS = BPC * HW                 # moving free dim per core = 4096
M_OUT = NSTACK * SIZE           # output channels = 2048
P = 128
KC = SIZE // P                  # contraction chunks = 4
MT = M_OUT // P                 # output row tiles = 16
NFREE = 512                     # matmul moving free dim (1 PSUM bank fp32)
NT = COLS // NFREE              # moving chunks = 8
GN = 4                          # psum tiles per group (half of PSUM banks)
NG = NT // GN                   # groups per m-tile = 2

DT_KIND = "f32r"

_CACHE = {}


def _build_nc(dt_kind):
    import concourse.bacc as bacc
    import concourse.tile as tile
    from concourse import mybir

    io_dt = {"bf16": mybir.dt.bfloat16,
             "f32r": mybir.dt.float32r,
             "f32": mybir.dt.float32}[dt_kind]

    nc = bacc.Bacc("TRN2", name="circulant1x1")
    x = nc.dram_tensor("x", [SIZE, COLS], io_dt, kind="ExternalInput")
    w = nc.dram_tensor("w", [SIZE, M_OUT], io_dt, kind="ExternalInput")
    out = nc.dram_tensor("out", [M_OUT, COLS], mybir.dt.float32,
                         kind="ExternalOutput")

    with tile.TileContext(nc) as tc:
        with (
            tc.tile_pool(name="xin", bufs=1) as xp,
            tc.tile_pool(name="win", bufs=1) as wp,
            tc.tile_pool(name="outp", bufs=4) as op,
            tc.tile_pool(name="ps", bufs=8, space="PSUM") as pp,
        ):
            HCOL = COLS // NG                   # columns per group = 2048
            x_sb = xp.tile([P, KC, COLS], io_dt)
            w_sb = wp.tile([P, KC, M_OUT], io_dt)

            # Input DMAs on the Sync HWDGE queue (outputs go on Scalar's),
            # ordered to match consumption: w chunk k interleaved with x's
            # group-0 half, then x's group-1 half. 12 DMAs stays under the
            # HWDGE in-flight cap (~10) without late-issue stalls.
            for k in range(KC):
                nc.sync.dma_start(out=w_sb[:, k, :],
                                  in_=w[k * P:(k + 1) * P, :])
                nc.sync.dma_start(out=x_sb[:, k, 0:HCOL],
                                  in_=x[k * P:(k + 1) * P, 0:HCOL])
            for k in range(KC):
                nc.sync.dma_start(out=x_sb[:, k, HCOL:COLS],
                                  in_=x[k * P:(k + 1) * P, HCOL:COLS])

            # HAM warmup: dummy matmuls on the first weight chunk while the
            # inputs stream in, so the PE is at K=8/8 (2.4 GHz) when the
            # real matmuls begin. Results discarded.
            for i in range(12):
                wps = pp.tile([P, NFREE], mybir.dt.float32, tag="ps",
                              name=f"warm_{i}")
                nc.tensor.matmul(wps, w_sb[:, 0, 0:P], w_sb[:, 0, 0:NFREE],
                                 start=True, stop=True)

            def copy_out(j, dst, src):
                if j % 2 == 0:
                    nc.vector.tensor_copy(out=dst, in_=src)
                else:
                    nc.scalar.copy(out=dst, in_=src)

            MR = 8  # ramp m-tiles
            # Phase 1 (ramp): column block j0 for m-tiles 0..7, k-outer, so
            # every arriving x group-0 chunk immediately feeds 8 matmuls
            # across all 8 PSUM banks instead of 1.
            ps1 = [pp.tile([P, NFREE], mybir.dt.float32, tag="ps",
                           name=f"ps1_{mi}") for mi in range(MR)]
            for k in range(KC):
                for mi in range(MR):
                    nc.tensor.matmul(ps1[mi], w_sb[:, k, mi * P:(mi + 1) * P],
                                     x_sb[:, k, 0:NFREE],
                                     start=(k == 0), stop=(k == KC - 1))
            for mi in range(MR):
                o1 = op.tile([P, NFREE], mybir.dt.float32, tag="osb0",
                             name=f"osb0_{mi}")
                copy_out(mi, o1[:], ps1[mi])
                nc.scalar.dma_start(out=out[mi * P:(mi + 1) * P, 0:NFREE],
                                    in_=o1[:])

            def do_group(m, g, js, tag):
                cols0 = (g * GN + js[0]) * NFREE
                width = len(js) * NFREE
                o_sb = op.tile([P, width], mybir.dt.float32, tag=tag,
                               name=f"osb_{m}_{g}_{js[0]}")
                ps = [pp.tile([P, NFREE], mybir.dt.float32, tag="ps",
                              name=f"ps_{m}_{g}_{j}") for j in js]
                for i, j in enumerate(js):
                    col = (g * GN + j) * NFREE
                    for k in range(KC):
                        nc.tensor.matmul(ps[i], w_sb[:, k, m * P:(m + 1) * P],
                                         x_sb[:, k, col:col + NFREE],
                                         start=(k == 0), stop=(k == KC - 1))
                for i, j in enumerate(js):
                    copy_out(j, o_sb[:, i * NFREE:(i + 1) * NFREE], ps[i])
                nc.scalar.dma_start(
                    out=out[m * P:(m + 1) * P, cols0:cols0 + width],
                    in_=o_sb[:])

            # Phase 2a: remaining group-0 column blocks for the ramp
            # m-tiles (same x half as phase 1 — no new input needed), then
            # 2b: their group-1 blocks, then phase 3: remaining m-tiles.
            for m in range(MR):
                do_group(m, 0, [1, 2, 3], "osbA")
            for m in range(MR):
                do_group(m, 1, [0, 1, 2, 3], "osb")
            for m in range(MR, MT):
                for g in range(NG):
                    do_group(m, g, [0, 1, 2, 3], "osb")
    nc.compile()
    return nc


def get_nc(dt_kind=DT_KIND):
    if dt_kind not in _CACHE:
        _CACHE[dt_kind] = _build_nc(dt_kind)
    return _CACHE[dt_kind]


def build_weight(c_f):
    """(NSTACK, SIZE//2+1, 2) rfft coeffs -> circulant weight W (SIZE, M_OUT),
    W[k, s*SIZE + n] = c_s[(n - k) mod SIZE]."""
    c_f = np.asarray(c_f, np.float32)
    cf = c_f[..., 0].astype(np.float64) + 1j * c_f[..., 1].astype(np.float64)
    c = np.fft.irfft(cf, n=SIZE, axis=-1)            # (NSTACK, SIZE) float64
    idx = (np.arange(SIZE)[None, :] - np.arange(SIZE)[:, None]) % SIZE
    W = np.empty((SIZE, M_OUT), np.float32)
    for s in range(NSTACK):
        W[:, s * SIZE:(s + 1) * SIZE] = c[s][idx]
    return W


def _round_fp32r(a):
    """RNE-round fp32 to the fp32r storage format (e8m11 in the high 20
    bits of the word) — what the PE consumes in fp32r matmul mode."""
    u = np.ascontiguousarray(a, np.float32).view(np.uint32).copy()
    u += 0x7FF + ((u >> 12) & 1)
    u &= 0xFFFFF000
    return u.view(np.float32)


def make_in_maps(x, c_f, dt_kind=DT_KIND):
    x = np.asarray(x, np.float32)
    W = build_weight(c_f)
    if dt_kind == "bf16":
        import ml_dtypes
        cast = lambda a: np.ascontiguousarray(a).astype(ml_dtypes.bfloat16)
    elif dt_kind == "f32r":
        cast = _round_fp32r
    else:
        cast = lambda a: np.ascontiguousarray(a, np.float32)
    Wc = cast(W)
    in_maps = []
    for i in range(N_CORES):
        xs = (x[i * BPC:(i + 1) * BPC]
              .reshape(BPC, SIZE, HW)
              .transpose(1, 0, 2)
              .reshape(SIZE, COLS))
        in_maps.append({"x": cast(xs), "w": Wc})
    return in_maps


def assemble_output(per_core_outs):
    """list of 8 (M_OUT, COLS) fp32 -> (BATCH, M_OUT, 32, 32) fp32"""
    parts = [o.reshape(M_OUT, BPC, HW).transpose(1, 0, 2)
             for o in per_core_outs]
    out = np.concatenate(parts, axis=0)               # (BATCH, M_OUT, HW)
    return np.ascontiguousarray(out.reshape(BATCH, M_OUT, 32, 32), np.float32)


def run(x, c_f, dt_kind=DT_KIND, **run_kwargs):
    """Returns (full_output, BassKernelResults)."""
    from concourse.bass_utils import run_bass_kernel_spmd
    nc = get_nc(dt_kind)
    in_maps = make_in_maps(x, c_f, dt_kind)
    res = run_bass_kernel_spmd(nc, in_maps, core_ids=list(range(N_CORES)),
                               **run_kwargs)
    out = assemble_output([r["out"] for r in res.results])
    return out, res


def kernel(input, c_f):
    out, _ = run(input, c_f)
    return out


# revision 12
# speedup vs baseline: 1.0035x; 1.0035x over previous
"""Circulant 1x1 conv (nn_Circulant1x1Conv) as a Trainium2 Bass kernel.

Math: the reference does, per spatial position r (N = batch*h*w rows):
    y[r, s*C + n] = irfft(rfft(x[r, :]) * cf[s])[n]  (circular convolution)
which is exactly a matmul  Y(N, 2048) = X(N, 512) @ W(512, 2048)  with
    W[k, s*C + n] = c_s[(n - k) mod C],   c_s = irfft(cf[s], n=C).

Crucially the native memory layouts are already transposed the right way:
  x[b] viewed as (C=512, h*w=1024) is X^T for that batch, and the output
  (nstack*C=2048, h*w) per batch is Y^T. So per batch:
      Out_b (2048, hw) = W^T @ X_b  ==  matmul(out, lhsT=W, rhs=X_b)
  on the tensor engine with zero data transposes anywhere.

Sharding: data-parallel over batch, 4 batches per core x 8 cores. Each core
computes a (2048, 4096) = (512, 2048)^T @ (512, 4096) matmul.

Precision knob DT_KIND:
  - "f32r": fp32 data, PE in fp32r (replicated/TF32-like) mode: 1 cycle/row
            at free-dim >= 256 per the cost model -> bf16-speed w/ fp32 inputs.
  - "bf16": inputs cast to bf16 on host; ~5e-3 rel error.
  - "f32":  exact fp32 matmul, 4 cycles/row (slow; debugging fallback).
"""

import numpy as np

SIZE = 512          # channels C (circulant size)
NSTACK = 4
BATCH = 32
HW = 32 * 32
N_CORES = 8
BPC = BATCH // N_CORES          # batches per core = 4
COLS = BPC * HW                 # moving free dim per core = 4096
M_OUT = NSTACK * SIZE           # output channels = 2048
P = 128
KC = SIZE // P                  # contraction chunks = 4
MT = M_OUT // P                 # output row tiles = 16
NFREE = 512                     # matmul moving free dim (1 PSUM bank fp32)
NT = COLS // NFREE              # moving chunks = 8
GN = 4                          # psum tiles per group (half of PSUM banks)
NG = NT // GN                   # groups per m-tile = 2

DT_KIND = "f32r"

_CACHE = {}


def _build_nc(dt_kind):
    import concourse.bacc as bacc
    import concourse.tile as tile
    from concourse import mybir

    io_dt = {"bf16": mybir.dt.bfloat16,
             "f32r": mybir.dt.float32r,
             "f32": mybir.dt.float32}[dt_kind]

    nc = bacc.Bacc("TRN2", name="circulant1x1")
    x = nc.dram_tensor("x", [SIZE, COLS], io_dt, kind="ExternalInput")
    w = nc.dram_tensor("w", [SIZE, M_OUT], io_dt, kind="ExternalInput")
    out = nc.dram_tensor("out", [M_OUT, COLS], mybir.dt.float32,
                         kind="ExternalOutput")

    with tile.TileContext(nc) as tc:
        with (
            tc.tile_pool(name="xin", bufs=1) as xp,
            tc.tile_pool(name="win", bufs=1) as wp,
            tc.tile_pool(name="outp", bufs=4) as op,
            tc.tile_pool(name="ps", bufs=8, space="PSUM") as pp,
        ):
            HCOL = COLS // NG                   # columns per group = 2048
            x_sb = xp.tile([P, KC, COLS], io_dt)
            w_sb = wp.tile([P, KC, M_OUT], io_dt)

            # Input DMAs on the Sync HWDGE queue (outputs go on Scalar's),
            # ordered to match consumption: w chunk k interleaved with x's
            # group-0 half, then x's group-1 half. 12 DMAs stays under the
            # HWDGE in-flight cap (~10) without late-issue stalls.
            for k in range(KC):
                nc.sync.dma_start(out=w_sb[:, k, :],
                                  in_=w[k * P:(k + 1) * P, :])
                nc.sync.dma_start(out=x_sb[:, k, 0:HCOL],
                                  in_=x[k * P:(k + 1) * P, 0:HCOL])
            for k in range(KC):
                nc.sync.dma_start(out=x_sb[:, k, HCOL:COLS],
                                  in_=x[k * P:(k + 1) * P, HCOL:COLS])

            # HAM warmup: dummy matmuls on the first weight chunk while the
            # inputs stream in, so the PE is at K=8/8 (2.4 GHz) when the
            # real matmuls begin. Results discarded.
            for i in range(12):
                wps = pp.tile([P, NFREE], mybir.dt.float32, tag="ps",
                              name=f"warm_{i}")
                nc.tensor.matmul(wps, w_sb[:, 0, 0:P], w_sb[:, 0, 0:NFREE],
                                 start=True, stop=True)

            ndummy = [0]

            def dummy_mms(n):
                # Filler matmuls on the first weight chunk: they have no
                # input dependencies, so in the PE's FIFO they run exactly
                # when the next real matmul would stall on an input DMA —
                # keeping the HAM busy-window alive (no 1.2 GHz re-throttle)
                # at ~227ns apiece. Results discarded.
                for _ in range(n):
                    i = ndummy[0]
                    ndummy[0] += 1
                    wps = pp.tile([P, NFREE], mybir.dt.float32, tag="ps",
                                  name=f"dummy_{i}")
                    nc.tensor.matmul(wps, w_sb[:, 0, 0:P],
                                     w_sb[:, 0, 0:NFREE],
                                     start=True, stop=True)

            # Graded dummy-fill for the input-arrival-bound ramp: the m0/m1
            # groups consume chunks as they land (~2.3us per MB) but only
            # have ~1us of real work per chunk.
            FILL = {(0, 0): 4, (0, 1): 4, (1, 0): 2, (1, 1): 2}

            for m in range(MT):
                for g in range(NG):
                    o_sb = op.tile([P, HCOL], mybir.dt.float32, tag="osb",
                                   name=f"osb_{m}_{g}")
                    ps = [pp.tile([P, NFREE], mybir.dt.float32, tag="ps",
                                  name=f"ps_{m}_{g}_{j}")
                          for j in range(GN)]
                    for j in range(GN):
                        col = (g * GN + j) * NFREE
                        for k in range(KC):
                            nc.tensor.matmul(ps[j], w_sb[:, k, m * P:(m + 1) * P],
                                             x_sb[:, k, col:col + NFREE],
                                             start=(k == 0), stop=(k == KC - 1))
                        dummy_mms(FILL.get((m, g), 0))
                    for j in range(GN):
                        if j % 2 == 0:
                            nc.vector.tensor_copy(
                                out=o_sb[:, j * NFREE:(j + 1) * NFREE],
                                in_=ps[j])
                        else:
                            nc.scalar.copy(
                                out=o_sb[:, j * NFREE:(j + 1) * NFREE],
                                in_=ps[j])
                    nc.scalar.dma_start(
                        out=out[m * P:(m + 1) * P, g * HCOL:(g + 1) * HCOL],
                        in_=o_sb[:])
    nc.compile()
    return nc


def get_nc(dt_kind=DT_KIND):
    if dt_kind not in _CACHE:
        _CACHE[dt_kind] = _build_nc(dt_kind)
    return _CACHE[dt_kind]


def build_weight(c_f):
    """(NSTACK, SIZE//2+1, 2) rfft coeffs -> circulant weight W (SIZE, M_OUT),
    W[k, s*SIZE + n] = c_s[(n - k) mod SIZE]."""
    c_f = np.asarray(c_f, np.float32)
    cf = c_f[..., 0].astype(np.float64) + 1j * c_f[..., 1].astype(np.float64)
    c = np.fft.irfft(cf, n=SIZE, axis=-1)            # (NSTACK, SIZE) float64
    idx = (np.arange(SIZE)[None, :] - np.arange(SIZE)[:, None]) % SIZE
    W = np.empty((SIZE, M_OUT), np.float32)
    for s in range(NSTACK):
        W[:, s * SIZE:(s + 1) * SIZE] = c[s][idx]
    return W


def _round_fp32r(a):
    """RNE-round fp32 to the fp32r storage format (e8m11 in the high 20
    bits of the word) — what the PE consumes in fp32r matmul mode."""
    u = np.ascontiguousarray(a, np.float32).view(np.uint32).copy()
    u += 0x7FF + ((u >> 12) & 1)
    u &= 0xFFFFF000
    return u.view(np.float32)


def make_in_maps(x, c_f, dt_kind=DT_KIND):
    x = np.asarray(x, np.float32)
    W = build_weight(c_f)
    if dt_kind == "bf16":
        import ml_dtypes
        cast = lambda a: np.ascontiguousarray(a).astype(ml_dtypes.bfloat16)
    elif dt_kind == "f32r":
        cast = _round_fp32r
    else:
        cast = lambda a: np.ascontiguousarray(a, np.float32)
    Wc = cast(W)
    in_maps = []
    for i in range(N_CORES):
        xs = (x[i * BPC:(i + 1) * BPC]
              .reshape(BPC, SIZE, HW)
              .transpose(1, 0, 2)
              .reshape(SIZE, COLS))
        in_maps.append({"x": cast(xs), "w": Wc})
    return in_maps


def assemble_output(per_core_outs):
    """list of 8 (M_OUT, COLS) fp32 -> (BATCH, M_OUT, 32, 32) fp32"""
    parts = [o.reshape(M_OUT, BPC, HW).transpose(1, 0, 2)
             for o in per_core_outs]
    out = np.concatenate(parts, axis=0)               # (BATCH, M_OUT, HW)
    return np.ascontiguousarray(out.reshape(BATCH, M_OUT, 32, 32), np.float32)


def run(x, c_f, dt_kind=DT_KIND, **run_kwargs):
    """Returns (full_output, BassKernelResults)."""
    from concourse.bass_utils import run_bass_kernel_spmd
    nc = get_nc(dt_kind)
    in_maps = make_in_maps(x, c_f, dt_kind)
    res = run_bass_kernel_spmd(nc, in_maps, core_ids=list(range(N_CORES)),
                               **run_kwargs)
    out = assemble_output([r["out"] for r in res.results])
    return out, res


def kernel(input, c_f):
    out, _ = run(input, c_f)
    return out
